# revision 25
# baseline (speedup 1.0000x reference)
"""Bilateral blur (7x7, sigma_color=0.1, sigma_space=1.5) on 8 trn2 NeuronCores.

Sharding: pure data-parallel, one image (3,512,512) per core.

Layout: each of the 128 partitions owns a 16-row x 128-col region of the
image (p = band*32 + strip). Per channel a partition stores a [28 x 144]
fp16 strip (6-row/6-col halo + alignment junk, owned base at (6,8)), so
every 7x7 tap shift is a pure free-dim offset. A +1-shifted copy (Xo/Wo)
provides 4B-aligned access for odd column shifts (DVE 2x perf mode).

Symmetry: |x(p+t)-x(p)| maps give w_t; the mirrored tap -t reuses the same
map: w_{-t}(p) = w_t(p-t). Only 24 tap-pairs are computed, each on the
union domain owned U owned-t.

Engine split per pair:
  DVE : 3 subtracts, 2 channel-sum adds, 6 weight*pixel mults, acc2 adds
  ACT : 3 abs, Square(sqrt(50)*a), Exp(-g + ln s_t)
  PE  : acc0/acc1 accumulation as identity matmuls into PSUM (f32)
  Pool: wsum accumulation
Center tap is the init (w = s_center exactly: scaled-identity matmul / TS).
Finale: out_c = acc_c * reciprocal(wsum) in fp32, scattered to (3,512,512).
"""

import sys

sys.path.insert(0, "/opt/trn_rl_repo")

import math

import numpy as np

import concourse.bass as bass
import concourse.mybir as mybir
from concourse import tile
from concourse.vector_clock import ScopedClock, VectorClock

# ---------------------------------------------------------------- constants
B, C, H, W = 8, 3, 512, 512
PAD = 3
COLOR_COEF = 50.0  # 0.5 / sigma_color^2

N_BANDS = 4
N_STRIPS = 32
ROWS = 16
COLS = 128
SR = 28  # stored rows: owned rows -6 .. +21
SC = 144  # stored cols: owned cols -8 .. +135
OR0 = 6  # stored row of owned row 0
OC0 = 8  # stored col of owned col 0
NFLAT = SR * SC  # 4032
NOWN = ROWS * COLS  # 2048
MAXW = 2560  # >= (16+3) * 134

F16 = mybir.dt.float16
F32 = mybir.dt.float32

POOL_WSUM = True  # wsum accumulation on the (otherwise idle) Pool engine


def _gaussian1d(ksize, sigma):
    x = np.arange(ksize, dtype=np.float64) - ksize // 2
    g = np.exp(-(x * x) / (2.0 * sigma * sigma))
    return g / g.sum()


_SY = _gaussian1d(7, 1.5)
_S2D = np.outer(_SY, _SY)
S_CENTER = float(_S2D[3, 3])

# 24 representative taps; the mirror -t reuses each w map
PAIRS = [
    (dy, dx)
    for dy in range(0, PAD + 1)
    for dx in range(-PAD, PAD + 1)
    if (dy > 0) or (dy == 0 and dx > 0)
]
assert len(PAIRS) == 24


def _space_w(dy, dx):
    return float(_S2D[dy + 3, dx + 3])


# ------------------------------------------------- tile tail-drain wait fix
# The installed walrus rejects >2 sync-wait commands on one CTRL instruction;
# spread the kernel-tail drain waits across per-proc NOPs (sequential on the
# same engine, so the drain still observes every semaphore target).
def _drain_and_barrier_split(self, tick_clock, wait_clock):
    nc = self.nc
    gc = tick_clock.global_clock
    n = len(gc)
    for proc in range(n):
        if gc[proc] <= 0:
            continue
        vc = VectorClock([gc[p] if p == proc else 0 for p in range(n)])
        nop = nc.sync.nop(nofuse=True, hint=f"tail_wait_p{proc}")
        wait_clock.add_sem_waits(nop.ins, ScopedClock({None: vc}))
    nc.sync.drain()
    nc.all_engine_barrier()
    assert self.sems is not None
    popped = nc._tile_sem_poison_stack.pop()
    assert popped is self._sem_poison
    nc.clear_and_free_semaphores(list(self.sems.allocated().values()))
    nc.all_engine_barrier()


tile.TileContext._drain_and_barrier = _drain_and_barrier_split


def _strip_redundant_self_waits(nc):
    """Remove sem waits that are trivially satisfied by the waiting engine's
    own program order (walrus allows only 1 sync-wait on ACT instructions).

    A wait (S, v) on an instruction of engine E is removable when S is
    incremented exclusively by earlier engine-E instructions whose cumulative
    increment already reaches v by this point in program order."""
    seq = []
    for fn in nc.m.functions:
        for bb in fn.blocks:
            seq.extend(bb.instructions)

    inc_engines = {}
    for ins in seq:
        si = getattr(ins, "sync_info", None)
        if si is None:
            continue
        for u in si.on_update or []:
            inc_engines.setdefault(u.ant_name, set()).add(str(ins.engine))

    cum = {}
    removed = 0
    for ins in seq:
        si = getattr(ins, "sync_info", None)
        if si is None:
            continue
        eng = str(ins.engine)
        ow = si.on_wait or []
        keep = []
        for w in ow:
            engs = inc_engines.get(w.ant_name)
            if (
                engs == {eng}
                and w.wait_mode in ("sem-ge-imm", "sem-ge")
                and cum.get(w.ant_name, 0) >= (w.wait_value or 0)
            ):
                removed += 1
                continue
            keep.append(w)
        if len(keep) != len(ow):
            si.on_wait = keep
        for u in si.on_update or []:
            cum[u.ant_name] = cum.get(u.ant_name, 0) + (u.update_value or 1)
    return removed


def _cap_waits(nc, limit=1):
    """This walrus build encodes at most `limit` sync-waits per instruction.
    Move excess waits backward onto an earlier same-engine instruction j.
    Safe when pos(j) > pos(producer of the waited sem value): the bb order
    is a topological order of the dep DAG, so the producer's transitive
    deps all precede it — no cycle can form; the engine is in-order, so
    waiting at j still gates the original instruction."""
    for fn in nc.m.functions:
        for bb in fn.blocks:
            seq = list(bb.instructions)
            # position where each sem's cumulative inc first reaches a value
            cum = {}
            reach = []  # per position: dict sem -> cum value AFTER this inst
            for ins in seq:
                si = getattr(ins, "sync_info", None)
                if si is not None:
                    for u in si.on_update or []:
                        cum[u.ant_name] = cum.get(u.ant_name, 0) + (
                            u.update_value or 1
                        )
                reach.append(dict(cum))

            def produced_pos(sem, val):
                # first position whose cumulative inc >= val; -1 if already
                # satisfied before this bb
                if val <= 0:
                    return -1
                for p, c in enumerate(reach):
                    if c.get(sem, 0) >= val:
                        return p
                return -1  # producer in an earlier bb

            nwaits = []
            for ins in seq:
                si = getattr(ins, "sync_info", None)
                nwaits.append(len(si.on_wait) if si is not None else 0)

            for i, ins in enumerate(seq):
                si = getattr(ins, "sync_info", None)
                if si is None or nwaits[i] <= limit:
                    continue
                ow = list(si.on_wait)
                # keep the wait with the latest producer (most binding)
                pp = [produced_pos(w.ant_name, w.wait_value or 0) for w in ow]
                order = sorted(range(len(ow)), key=lambda k: pp[k])
                keep_idx = order[-1]
                moved_any = []
                for k in order[:-1]:
                    if len(moved_any) + 1 >= len(ow) - (limit - 1):
                        break  # moved enough
                    w, q = ow[k], pp[k]
                    j = i - 1
                    dst = None
                    while j > q:
                        o = seq[j]
                        osi = getattr(o, "sync_info", None)
                        if (
                            str(o.engine) == str(ins.engine)
                            and osi is not None
                            and nwaits[j] < limit
                        ):
                            dst = j
                            break
                        j -= 1
                    if dst is None:
                        continue
                    osi = seq[dst].sync_info
                    osi.on_wait = list(osi.on_wait) + [w]
                    nwaits[dst] += 1
                    moved_any.append(k)
                if moved_any:
                    si.on_wait = [
                        w for k, w in enumerate(ow) if k not in moved_any
                    ]
                    nwaits[i] = len(si.on_wait)
                if nwaits[i] > limit:
                    raise RuntimeError(
                        f"could not cap waits on {ins.name}: "
                        f"{[(w.ant_name, w.wait_value) for w in si.on_wait]}"
                    )


# ------------------------------------------------------------- host helpers
def _host_prep_core(img):
    """img: (3, 512, 512) f32 -> strips [128, 3, 28, 144] f16."""
    padded = np.pad(img, ((0, 0), (8, 8), (8, 8)), mode="reflect").astype(np.float16)
    out = np.empty((128, C, SR, SC), np.float16)
    for b in range(N_BANDS):
        for s in range(N_STRIPS):
            p = b * N_STRIPS + s
            out[p] = padded[:, 16 * s + 2 : 16 * s + 2 + SR, 128 * b : 128 * b + SC]
    return out


_IDENT = None


def _host_ident():
    global _IDENT
    if _IDENT is None:
        e = np.eye(128, dtype=np.float16)
        _IDENT = np.concatenate([e, -e], 1)
    return _IDENT


def _v3(t, r0, nr, c0, ncols, w):
    ap = t[:]
    nfit = (ap.shape[-1] // w) * w
    if nfit != ap.shape[-1]:
        ap = ap[:, 0:nfit]
    return ap.rearrange("p (r w) -> p r w", w=w)[:, r0 : r0 + nr, c0 : c0 + ncols]


# ------------------------------------------------------------ program build
def build_program():
    nc = bass.Bass()
    xs_in = nc.dram_tensor("xs", [128, C, NFLAT], F16, kind="ExternalInput")
    id_in = nc.dram_tensor("ident", [128, 256], F16, kind="ExternalInput")
    out_d = nc.dram_tensor(
        "out", [C, N_BANDS, N_STRIPS, ROWS, COLS], F32, kind="ExternalOutput"
    )

    alu = mybir.AluOpType
    act = mybir.ActivationFunctionType

    with tile.TileContext(nc) as tc:
        with (
            tc.tile_pool(name="xp", bufs=1) as xp,
            tc.tile_pool(name="wk", bufs=2) as wk,
            tc.tile_pool(name="wp", bufs=3) as wp,
            tc.tile_pool(name="ac", bufs=1) as ac,
            tc.tile_pool(name="ps", bufs=1, space="PSUM") as ps,
        ):
            X = [
                xp.tile([128, NFLAT], F16, tag=f"X{c}", name=f"X{c}") for c in range(C)
            ]
            Xo = [
                xp.tile([128, NFLAT], F16, tag=f"Xo{c}", name=f"Xo{c}")
                for c in range(C)
            ]
            idt = xp.tile([128, 256], F16, tag="idt", name="idt")
            nc.sync.dma_start(idt[:], id_in[:])
            for c in range(C):
                nc.sync.dma_start(X[c][:], xs_in[:, c, :])
            for c in range(C):
                nc.vector.tensor_copy(Xo[c][:, 0 : NFLAT - 1], X[c][:, 1:NFLAT])

            ident = idt[:, 0:128]
            ident_neg = idt[:, 128:256]

            acc2 = ac.tile([128, NOWN], F16, tag="acc2", name="acc2")
            wsum = ac.tile([128, NOWN], F16, tag="wsum", name="wsum")
            pacc = [
                ps.tile([128, NOWN], F32, tag=f"pacc{c}", name=f"pacc{c}")
                for c in range(2)
            ]

            def xsl(c, r0, c0, nr, ncols):
                """strip view at stored rows r0.., cols c0.. (image offsets
                relative to owned origin are handled by callers)."""
                if c0 % 2 == 0:
                    return _v3(X[c], r0, nr, c0, ncols, SC)
                return _v3(Xo[c], r0, nr, c0 - 1, ncols, SC)

            def own(t):  # dense [128,16,128] view of an owned-domain tile
                return _v3(t, 0, ROWS, 0, COLS, COLS)

            # deviation form: n_c = sum_k w_k * (x_c(p+k) - x_c(p)); the
            # center tap contributes nothing, so accumulators start at 0
            # (first matmul start=True; acc2 memset). wsum is split into two
            # independent accumulators (DVE + Pool chains) merged at the end.
            nc.vector.memset(acc2[:], 0.0)
            nc.vector.memset(wsum[:], S_CENTER)
            wsumB = ac.tile([128, NOWN], F16, tag="wsumB", name="wsumB")
            if POOL_WSUM:
                nc.gpsimd.memset(wsumB[:], 0.0)

            # per-pair Exp biases (ln of space weight), via a const fp32 tile
            bias_vals = sorted(
                {round(math.log(_space_w(dy, dx)), 12) for dy, dx in PAIRS}
            )
            bias_idx = {v: i for i, v in enumerate(bias_vals)}
            bias_t = ac.tile([128, len(bias_vals)], F32, tag="bias", name="bias_t")
            for i, v in enumerate(bias_vals):
                nc.vector.memset(bias_t[:, i : i + 1], v)

            def produce_subs(dy, dx):
                """diffs + abs for pair (dy,dx); returns pipeline context."""
                adx = abs(dx)
                qr0, qc0 = -dy, -max(dx, 0)
                qrows, qcols = ROWS + dy, COLS + adx
                wpad = qcols + (qcols & 1)  # even width for 2x alignment

                def wv(t):
                    return _v3(t, 0, qrows, 0, wpad, wpad)

                sr0, sc0 = OR0 + qr0, OC0 + qc0
                ds, abs_ = [], []
                for c in range(C):
                    d = wk.tile(
                        [128, MAXW], F16, tag=f"d{c}", name=f"d{c}", bufs=3
                    )
                    nc.vector.tensor_tensor(
                        wv(d),
                        xsl(c, sr0 + dy, sc0 + dx, qrows, wpad),
                        xsl(c, sr0, sc0, qrows, wpad),
                        alu.subtract,
                    )
                    ab = wk.tile([128, MAXW], F16, tag=f"ab{c}", name=f"ab{c}")
                    nc.scalar.activation(wv(ab), wv(d), act.Abs)
                    ds.append(d)
                    abs_.append(ab)
                return [dy, dx, wpad, ds, abs_, wv, None]

            def produce_rest(ctx):
                """channel-sum + square + exp for a pair."""
                dy, dx, wpad, ds, abs_, wv, _ = ctx
                e = wk.tile([128, MAXW], F16, tag="e", name="e")
                nc.vector.tensor_tensor(wv(e), wv(abs_[0]), wv(abs_[1]), alu.add)
                nc.vector.tensor_tensor(wv(e), wv(e), wv(abs_[2]), alu.add)
                nc.scalar.activation(
                    wv(e), wv(e), act.Square, scale=math.sqrt(COLOR_COEF)
                )
                wt = wp.tile([128, MAXW], F16, tag="wt", name="wt")
                bi = bias_idx[round(math.log(_space_w(dy, dx)), 12)]
                nc.scalar.activation(
                    wv(wt), wv(e), act.Exp, bias=bias_t[:, bi : bi + 1], scale=-1.0
                )
                ctx[6] = wt

            def consume_mults(ctx, first, last):
                dy, dx, wpad, ds, abs_, wv, wt = ctx
                cA, cB = max(dx, 0), max(-dx, 0)
                # m_c = w_t * d_c in-place on d_c; read at (dy,cA) for +t and
                # negated at (0,cB) for -t
                for c in range(C):
                    nc.vector.tensor_tensor(wv(ds[c]), wv(ds[c]), wv(wt), alu.mult)

                def mview(c, r0, c0, k):
                    return _v3(ds[c], r0 + 4 * k, 4, c0, COLS, wpad)

                for c in range(2):
                    for k in range(4):
                        sl = slice(512 * k, 512 * (k + 1))
                        nc.tensor.matmul(
                            pacc[c][:, sl], ident, mview(c, dy, cA, k),
                            start=first, stop=False, skip_group_check=True,
                        )
                        nc.tensor.matmul(
                            pacc[c][:, sl], ident_neg, mview(c, 0, cB, k),
                            start=False, stop=last, skip_group_check=True,
                        )

            def consume_rest(ctx):
                dy, dx, wpad, ds, abs_, wv, wt = ctx
                cA, cB = max(dx, 0), max(-dx, 0)
                m2p = _v3(ds[2], dy, ROWS, cA, COLS, wpad)
                m2m = _v3(ds[2], 0, ROWS, cB, COLS, wpad)
                nc.vector.tensor_tensor(own(acc2), own(acc2), m2p, alu.add)
                nc.vector.tensor_tensor(own(acc2), own(acc2), m2m, alu.subtract)

                wA = _v3(wt, dy, ROWS, cA, COLS, wpad)
                wB = _v3(wt, 0, ROWS, cB, COLS, wpad)
                if POOL_WSUM:
                    nc.gpsimd.tensor_tensor(own(wsumB), own(wsumB), wA, alu.add)
                    nc.gpsimd.tensor_tensor(own(wsumB), own(wsumB), wB, alu.add)
                else:
                    nc.vector.tensor_tensor(own(wsum), own(wsum), wA, alu.add)
                    nc.vector.tensor_tensor(own(wsum), own(wsum), wB, alu.add)

            # software pipeline, interleaved so ACT's sq/exp of pair i runs
            # under the DVE subs of pair i+1, and the DVE m-mults of pair i
            # run under ACT's abs of pair i+1
            prev = None
            for pi, (dy, dx) in enumerate(PAIRS):
                cur = produce_subs(dy, dx)
                if prev is not None:
                    consume_mults(prev, first=(pi == 1), last=False)
                produce_rest(cur)
                if prev is not None:
                    consume_rest(prev)
                prev = cur
            consume_mults(prev, first=False, last=True)
            consume_rest(prev)

            # ---- finale: out_c = x_c + n_c * (1 / wsum) ----
            if POOL_WSUM:
                nc.vector.tensor_tensor(wsum[:], wsum[:], wsumB[:], alu.add)
            rec = ac.tile([128, NOWN], F32, tag="rec", name="rec")
            nc.vector.reciprocal(rec[:], wsum[:])
            outs = []
            for c in range(C):
                o = ac.tile([128, NOWN], F32, tag="o", name=f"oo{c}", bufs=3)
                src = pacc[c][:] if c < 2 else acc2[:]
                nc.vector.tensor_tensor(o[:], src, rec[:], alu.mult)
                nc.vector.tensor_tensor(
                    own(o), own(o), xsl(c, OR0, OC0, ROWS, COLS), alu.add
                )
                outs.append(o)
            for c in range(C):
                nc.sync.dma_start(
                    out_d[c].rearrange("b s r w -> (b s) (r w)"), outs[c][:]
                )
    return nc


_NC_CACHE = None


def _get_nc():
    global _NC_CACHE
    if _NC_CACHE is None:
        nc = build_program()
        # walrus in this image encodes at most 1 sync-wait per instruction;
        # rely on same-engine program order instead (safe on in-order engines)
        _strip_redundant_self_waits(nc)
        _cap_waits(nc, limit=1)
        _NC_CACHE = nc
    return _NC_CACHE


# ---------------------------------------------------------------- interface
def _install_ntff_hook():
    """The image's antenv lacks axon_hooks; synthesize it so trace=True works."""
    import sys as _sys
    import types

    if "antenv.axon_hooks" in _sys.modules:
        return
    mod = types.ModuleType("antenv.axon_hooks")
    mod._hook = None

    def set_axon_ntff_profile_hook(h):
        mod._hook = h

    def get_axon_ntff_profile_hook():
        return mod._hook

    mod.set_axon_ntff_profile_hook = set_axon_ntff_profile_hook
    mod.get_axon_ntff_profile_hook = get_axon_ntff_profile_hook
    _sys.modules["antenv.axon_hooks"] = mod
    import antenv

    antenv.axon_hooks = mod
    _sys.path.insert(0, "/root/.axon_site")
    from trn_agent_boot.trn_boot import _ntff_profile_via_ctypes

    h = _ntff_profile_via_ctypes("/opt/axon/libaxon_pjrt.so")
    if h is not None:
        set_axon_ntff_profile_hook(h)

    from concourse import bass_utils as _bu

    _bu.upload_artifacts = lambda tmpdir: tmpdir


def _run(inputs_full, trace=False):
    from concourse.bass_utils import run_bass_kernel_spmd

    if trace:
        _install_ntff_hook()

    x = np.asarray(inputs_full, np.float32)
    ident = _host_ident()
    in_maps = [
        {"xs": _host_prep_core(x[i]).reshape(128, C, NFLAT), "ident": ident}
        for i in range(B)
    ]
    nc = _get_nc()
    res = run_bass_kernel_spmd(nc, in_maps, list(range(B)), trace=trace)
    out = np.stack(
        [
            res.results[i]["out"].transpose(0, 2, 3, 1, 4).reshape(C, H, W)
            for i in range(B)
        ],
        axis=0,
    )
    return out, res


def kernel(input):
    out, _ = _run(input, trace=False)
    return out


def kernel_profiled(input):
    return _run(input, trace=True)


# ----------------------------------------------------- local sim validation
def _sim_one(img):
    from concourse.bass_interp import CoreSim

    nc = build_program()
    sim = CoreSim(nc, trace=False)
    sim.tensor("xs")[:] = _host_prep_core(img).reshape(128, C, NFLAT)
    sim.tensor("ident")[:] = _host_ident()
    sim.simulate()
    return np.array(sim.tensor("out")).transpose(0, 2, 3, 1, 4).reshape(C, H, W)


if __name__ == "__main__":
    rng = np.random.default_rng(0)
    img = rng.random((C, H, W), np.float32)
    out = _sim_one(img)
    print("sim out stats", out.min(), out.max(), np.abs(out).mean())


# revision 26
# speedup vs baseline: 1.0024x; 1.0024x over previous
"""Bilateral blur (7x7, sigma_color=0.1, sigma_space=1.5) on 8 trn2 NeuronCores.

Sharding: pure data-parallel, one image (3,512,512) per core.

Layout: each of the 128 partitions owns a 16-row x 128-col region of the
image (p = band*32 + strip). Per channel a partition stores a [28 x 144]
fp16 strip (6-row/6-col halo + alignment junk, owned base at (6,8)), so
every 7x7 tap shift is a pure free-dim offset. A +1-shifted copy (Xo/Wo)
provides 4B-aligned access for odd column shifts (DVE 2x perf mode).

Symmetry: |x(p+t)-x(p)| maps give w_t; the mirrored tap -t reuses the same
map: w_{-t}(p) = w_t(p-t). Only 24 tap-pairs are computed, each on the
union domain owned U owned-t.

Engine split per pair:
  DVE : 3 subtracts, 2 channel-sum adds, 6 weight*pixel mults, acc2 adds
  ACT : 3 abs, Square(sqrt(50)*a), Exp(-g + ln s_t)
  PE  : acc0/acc1 accumulation as identity matmuls into PSUM (f32)
  Pool: wsum accumulation
Center tap is the init (w = s_center exactly: scaled-identity matmul / TS).
Finale: out_c = acc_c * reciprocal(wsum) in fp32, scattered to (3,512,512).
"""

import sys

sys.path.insert(0, "/opt/trn_rl_repo")

import math

import numpy as np

import concourse.bass as bass
import concourse.mybir as mybir
from concourse import tile
from concourse.vector_clock import ScopedClock, VectorClock

# ---------------------------------------------------------------- constants
B, C, H, W = 8, 3, 512, 512
PAD = 3
COLOR_COEF = 50.0  # 0.5 / sigma_color^2

N_BANDS = 4
N_STRIPS = 32
ROWS = 16
COLS = 128
SR = 28  # stored rows: owned rows -6 .. +21
SC = 144  # stored cols: owned cols -8 .. +135
OR0 = 6  # stored row of owned row 0
OC0 = 8  # stored col of owned col 0
NFLAT = SR * SC  # 4032
NOWN = ROWS * COLS  # 2048
MAXW = 2560  # >= (16+3) * 134

F16 = mybir.dt.float16
F32 = mybir.dt.float32

POOL_WSUM = True  # wsum accumulation on the (otherwise idle) Pool engine


def _gaussian1d(ksize, sigma):
    x = np.arange(ksize, dtype=np.float64) - ksize // 2
    g = np.exp(-(x * x) / (2.0 * sigma * sigma))
    return g / g.sum()


_SY = _gaussian1d(7, 1.5)
_S2D = np.outer(_SY, _SY)
S_CENTER = float(_S2D[3, 3])

# 24 representative taps; the mirror -t reuses each w map
PAIRS = [
    (dy, dx)
    for dy in range(0, PAD + 1)
    for dx in range(-PAD, PAD + 1)
    if (dy > 0) or (dy == 0 and dx > 0)
]
assert len(PAIRS) == 24


def _space_w(dy, dx):
    return float(_S2D[dy + 3, dx + 3])


# ------------------------------------------------- tile tail-drain wait fix
# The installed walrus rejects >2 sync-wait commands on one CTRL instruction;
# spread the kernel-tail drain waits across per-proc NOPs (sequential on the
# same engine, so the drain still observes every semaphore target).
def _drain_and_barrier_split(self, tick_clock, wait_clock):
    nc = self.nc
    gc = tick_clock.global_clock
    n = len(gc)
    for proc in range(n):
        if gc[proc] <= 0:
            continue
        vc = VectorClock([gc[p] if p == proc else 0 for p in range(n)])
        nop = nc.sync.nop(nofuse=True, hint=f"tail_wait_p{proc}")
        wait_clock.add_sem_waits(nop.ins, ScopedClock({None: vc}))
    nc.sync.drain()
    nc.all_engine_barrier()
    assert self.sems is not None
    popped = nc._tile_sem_poison_stack.pop()
    assert popped is self._sem_poison
    nc.clear_and_free_semaphores(list(self.sems.allocated().values()))
    nc.all_engine_barrier()


tile.TileContext._drain_and_barrier = _drain_and_barrier_split


def _strip_redundant_self_waits(nc):
    """Remove sem waits that are trivially satisfied by the waiting engine's
    own program order (walrus allows only 1 sync-wait on ACT instructions).

    A wait (S, v) on an instruction of engine E is removable when S is
    incremented exclusively by earlier engine-E instructions whose cumulative
    increment already reaches v by this point in program order."""
    seq = []
    for fn in nc.m.functions:
        for bb in fn.blocks:
            seq.extend(bb.instructions)

    inc_engines = {}
    for ins in seq:
        si = getattr(ins, "sync_info", None)
        if si is None:
            continue
        for u in si.on_update or []:
            inc_engines.setdefault(u.ant_name, set()).add(str(ins.engine))

    cum = {}
    removed = 0
    for ins in seq:
        si = getattr(ins, "sync_info", None)
        if si is None:
            continue
        eng = str(ins.engine)
        ow = si.on_wait or []
        keep = []
        for w in ow:
            engs = inc_engines.get(w.ant_name)
            if (
                engs == {eng}
                and w.wait_mode in ("sem-ge-imm", "sem-ge")
                and cum.get(w.ant_name, 0) >= (w.wait_value or 0)
            ):
                removed += 1
                continue
            keep.append(w)
        if len(keep) != len(ow):
            si.on_wait = keep
        for u in si.on_update or []:
            cum[u.ant_name] = cum.get(u.ant_name, 0) + (u.update_value or 1)
    return removed


def _cap_waits(nc, limit=1):
    """This walrus build encodes at most `limit` sync-waits per instruction.
    Move excess waits backward onto an earlier same-engine instruction j.
    Safe when pos(j) > pos(producer of the waited sem value): the bb order
    is a topological order of the dep DAG, so the producer's transitive
    deps all precede it — no cycle can form; the engine is in-order, so
    waiting at j still gates the original instruction."""
    for fn in nc.m.functions:
        for bb in fn.blocks:
            seq = list(bb.instructions)
            # position where each sem's cumulative inc first reaches a value
            cum = {}
            reach = []  # per position: dict sem -> cum value AFTER this inst
            for ins in seq:
                si = getattr(ins, "sync_info", None)
                if si is not None:
                    for u in si.on_update or []:
                        cum[u.ant_name] = cum.get(u.ant_name, 0) + (
                            u.update_value or 1
                        )
                reach.append(dict(cum))

            def produced_pos(sem, val):
                # first position whose cumulative inc >= val; -1 if already
                # satisfied before this bb
                if val <= 0:
                    return -1
                for p, c in enumerate(reach):
                    if c.get(sem, 0) >= val:
                        return p
                return -1  # producer in an earlier bb

            nwaits = []
            for ins in seq:
                si = getattr(ins, "sync_info", None)
                nwaits.append(len(si.on_wait) if si is not None else 0)

            for i, ins in enumerate(seq):
                si = getattr(ins, "sync_info", None)
                if si is None or nwaits[i] <= limit:
                    continue
                ow = list(si.on_wait)
                # keep the wait with the latest producer (most binding)
                pp = [produced_pos(w.ant_name, w.wait_value or 0) for w in ow]
                order = sorted(range(len(ow)), key=lambda k: pp[k])
                keep_idx = order[-1]
                moved_any = []
                for k in order[:-1]:
                    if len(moved_any) + 1 >= len(ow) - (limit - 1):
                        break  # moved enough
                    w, q = ow[k], pp[k]
                    j = i - 1
                    dst = None
                    while j > q:
                        o = seq[j]
                        osi = getattr(o, "sync_info", None)
                        if (
                            str(o.engine) == str(ins.engine)
                            and osi is not None
                            and nwaits[j] < limit
                        ):
                            dst = j
                            break
                        j -= 1
                    if dst is None:
                        continue
                    osi = seq[dst].sync_info
                    osi.on_wait = list(osi.on_wait) + [w]
                    nwaits[dst] += 1
                    moved_any.append(k)
                if moved_any:
                    si.on_wait = [
                        w for k, w in enumerate(ow) if k not in moved_any
                    ]
                    nwaits[i] = len(si.on_wait)
                if nwaits[i] > limit:
                    raise RuntimeError(
                        f"could not cap waits on {ins.name}: "
                        f"{[(w.ant_name, w.wait_value) for w in si.on_wait]}"
                    )


# ------------------------------------------------------------- host helpers
def _host_prep_core(img):
    """img: (3, 512, 512) f32 -> strips [128, 3, 28, 144] f16."""
    padded = np.pad(img, ((0, 0), (8, 8), (8, 8)), mode="reflect").astype(np.float16)
    out = np.empty((128, C, SR, SC), np.float16)
    for b in range(N_BANDS):
        for s in range(N_STRIPS):
            p = b * N_STRIPS + s
            out[p] = padded[:, 16 * s + 2 : 16 * s + 2 + SR, 128 * b : 128 * b + SC]
    return out


_IDENT = None


def _host_ident():
    global _IDENT
    if _IDENT is None:
        e = np.eye(128, dtype=np.float16)
        _IDENT = np.concatenate([e, -e], 1)
    return _IDENT


def _v3(t, r0, nr, c0, ncols, w):
    ap = t[:]
    nfit = (ap.shape[-1] // w) * w
    if nfit != ap.shape[-1]:
        ap = ap[:, 0:nfit]
    return ap.rearrange("p (r w) -> p r w", w=w)[:, r0 : r0 + nr, c0 : c0 + ncols]


# ------------------------------------------------------------ program build
def build_program():
    nc = bass.Bass()
    xs_in = nc.dram_tensor("xs", [128, C, NFLAT], F16, kind="ExternalInput")
    id_in = nc.dram_tensor("ident", [128, 256], F16, kind="ExternalInput")
    out_d = nc.dram_tensor(
        "out", [C, N_BANDS, N_STRIPS, ROWS, COLS], F32, kind="ExternalOutput"
    )

    alu = mybir.AluOpType
    act = mybir.ActivationFunctionType

    with tile.TileContext(nc) as tc:
        with (
            tc.tile_pool(name="xp", bufs=1) as xp,
            tc.tile_pool(name="wk", bufs=2) as wk,
            tc.tile_pool(name="wp", bufs=3) as wp,
            tc.tile_pool(name="ac", bufs=1) as ac,
            tc.tile_pool(name="ps", bufs=1, space="PSUM") as ps,
        ):
            X = [
                xp.tile([128, NFLAT], F16, tag=f"X{c}", name=f"X{c}") for c in range(C)
            ]
            Xo = [
                xp.tile([128, NFLAT], F16, tag=f"Xo{c}", name=f"Xo{c}")
                for c in range(C)
            ]
            idt = xp.tile([128, 256], F16, tag="idt", name="idt")
            nc.sync.dma_start(idt[:], id_in[:])
            for c in range(C):
                nc.sync.dma_start(X[c][:], xs_in[:, c, :])
            for c in range(C):
                nc.vector.tensor_copy(Xo[c][:, 0 : NFLAT - 1], X[c][:, 1:NFLAT])

            ident = idt[:, 0:128]
            ident_neg = idt[:, 128:256]

            acc2 = ac.tile([128, NOWN], F16, tag="acc2", name="acc2")
            wsum = ac.tile([128, NOWN], F16, tag="wsum", name="wsum")
            pacc = [
                ps.tile([128, NOWN], F32, tag=f"pacc{c}", name=f"pacc{c}")
                for c in range(2)
            ]

            def xsl(c, r0, c0, nr, ncols):
                """strip view at stored rows r0.., cols c0.. (image offsets
                relative to owned origin are handled by callers)."""
                if c0 % 2 == 0:
                    return _v3(X[c], r0, nr, c0, ncols, SC)
                return _v3(Xo[c], r0, nr, c0 - 1, ncols, SC)

            def own(t):  # dense [128,16,128] view of an owned-domain tile
                return _v3(t, 0, ROWS, 0, COLS, COLS)

            # deviation form: n_c = sum_k w_k * (x_c(p+k) - x_c(p)); the
            # center tap contributes nothing, so accumulators start at 0
            # (first matmul start=True; acc2 memset). wsum is split into two
            # independent accumulators (DVE + Pool chains) merged at the end.
            nc.vector.memset(acc2[:], 0.0)
            nc.vector.memset(wsum[:], S_CENTER)
            wsumB = ac.tile([128, NOWN], F16, tag="wsumB", name="wsumB")
            if POOL_WSUM:
                nc.gpsimd.memset(wsumB[:], 0.0)

            # per-pair Exp biases (ln of space weight), via a const fp32 tile
            bias_vals = sorted(
                {round(math.log(_space_w(dy, dx)), 12) for dy, dx in PAIRS}
            )
            bias_idx = {v: i for i, v in enumerate(bias_vals)}
            bias_t = ac.tile([128, len(bias_vals)], F32, tag="bias", name="bias_t")
            for i, v in enumerate(bias_vals):
                nc.vector.memset(bias_t[:, i : i + 1], v)

            def produce_subs(dy, dx):
                """diffs + abs for pair (dy,dx); returns pipeline context."""
                adx = abs(dx)
                qr0, qc0 = -dy, -max(dx, 0)
                qrows, qcols = ROWS + dy, COLS + adx
                wpad = qcols + (qcols & 1)  # even width for 2x alignment

                def wv(t):
                    return _v3(t, 0, qrows, 0, wpad, wpad)

                sr0, sc0 = OR0 + qr0, OC0 + qc0
                ds, abs_ = [], []
                for c in range(C):
                    d = wk.tile(
                        [128, MAXW], F16, tag=f"d{c}", name=f"d{c}", bufs=3
                    )
                    nc.vector.tensor_tensor(
                        wv(d),
                        xsl(c, sr0 + dy, sc0 + dx, qrows, wpad),
                        xsl(c, sr0, sc0, qrows, wpad),
                        alu.subtract,
                    )
                    ab = wk.tile([128, MAXW], F16, tag=f"ab{c}", name=f"ab{c}")
                    nc.scalar.activation(wv(ab), wv(d), act.Abs)
                    ds.append(d)
                    abs_.append(ab)
                return [dy, dx, wpad, ds, abs_, wv, None]

            def produce_rest(ctx):
                """channel-sum + square + exp for a pair."""
                dy, dx, wpad, ds, abs_, wv, _ = ctx
                e = wk.tile([128, MAXW], F16, tag="e", name="e")
                nc.vector.tensor_tensor(wv(e), wv(abs_[0]), wv(abs_[1]), alu.add)
                nc.vector.tensor_tensor(wv(e), wv(e), wv(abs_[2]), alu.add)
                nc.scalar.activation(
                    wv(e), wv(e), act.Square, scale=math.sqrt(COLOR_COEF)
                )
                wt = wp.tile([128, MAXW], F16, tag="wt", name="wt")
                bi = bias_idx[round(math.log(_space_w(dy, dx)), 12)]
                nc.scalar.activation(
                    wv(wt), wv(e), act.Exp, bias=bias_t[:, bi : bi + 1], scale=-1.0
                )
                ctx[6] = wt

            def consume_mults(ctx, first, last):
                dy, dx, wpad, ds, abs_, wv, wt = ctx
                cA, cB = max(dx, 0), max(-dx, 0)
                # m_c = w_t * d_c in-place on d_c; read at (dy,cA) for +t and
                # negated at (0,cB) for -t
                for c in range(C):
                    nc.vector.tensor_tensor(wv(ds[c]), wv(ds[c]), wv(wt), alu.mult)

                def mview(c, r0, c0, k):
                    return _v3(ds[c], r0 + 4 * k, 4, c0, COLS, wpad)

                for c in range(2):
                    for k in range(4):
                        sl = slice(512 * k, 512 * (k + 1))
                        nc.tensor.matmul(
                            pacc[c][:, sl], ident, mview(c, dy, cA, k),
                            start=first, stop=False, skip_group_check=True,
                        )
                        nc.tensor.matmul(
                            pacc[c][:, sl], ident_neg, mview(c, 0, cB, k),
                            start=False, stop=last, skip_group_check=True,
                        )

            def consume_rest(ctx):
                dy, dx, wpad, ds, abs_, wv, wt = ctx
                cA, cB = max(dx, 0), max(-dx, 0)
                m2p = _v3(ds[2], dy, ROWS, cA, COLS, wpad)
                m2m = _v3(ds[2], 0, ROWS, cB, COLS, wpad)
                nc.vector.tensor_tensor(own(acc2), own(acc2), m2p, alu.add)
                nc.vector.tensor_tensor(own(acc2), own(acc2), m2m, alu.subtract)

                wA = _v3(wt, dy, ROWS, cA, COLS, wpad)
                wB = _v3(wt, 0, ROWS, cB, COLS, wpad)
                if POOL_WSUM:
                    nc.gpsimd.tensor_tensor(own(wsumB), own(wsumB), wA, alu.add)
                    nc.gpsimd.tensor_tensor(own(wsumB), own(wsumB), wB, alu.add)
                else:
                    nc.vector.tensor_tensor(own(wsum), own(wsum), wA, alu.add)
                    nc.vector.tensor_tensor(own(wsum), own(wsum), wB, alu.add)

            # software pipeline: produce pair i+1 fully, then consume pair i
            prev = None
            for pi, (dy, dx) in enumerate(PAIRS):
                cur = produce_subs(dy, dx)
                produce_rest(cur)
                if prev is not None:
                    consume_mults(prev, first=(pi == 1), last=False)
                    consume_rest(prev)
                prev = cur
            consume_mults(prev, first=False, last=True)
            consume_rest(prev)

            # ---- finale: out_c = x_c + n_c * (1 / wsum) ----
            if POOL_WSUM:
                nc.vector.tensor_tensor(wsum[:], wsum[:], wsumB[:], alu.add)
            rec = ac.tile([128, NOWN], F32, tag="rec", name="rec")
            nc.vector.reciprocal(rec[:], wsum[:])
            outs = []
            for c in range(C):
                o = ac.tile([128, NOWN], F32, tag="o", name=f"oo{c}", bufs=3)
                src = pacc[c][:] if c < 2 else acc2[:]
                nc.vector.tensor_tensor(o[:], src, rec[:], alu.mult)
                nc.vector.tensor_tensor(
                    own(o), own(o), xsl(c, OR0, OC0, ROWS, COLS), alu.add
                )
                outs.append(o)
            for c in range(C):
                nc.sync.dma_start(
                    out_d[c].rearrange("b s r w -> (b s) (r w)"), outs[c][:]
                )
    return nc


_NC_CACHE = None


def _get_nc():
    global _NC_CACHE
    if _NC_CACHE is None:
        nc = build_program()
        # walrus in this image encodes at most 1 sync-wait per instruction;
        # rely on same-engine program order instead (safe on in-order engines)
        _strip_redundant_self_waits(nc)
        _cap_waits(nc, limit=1)
        _NC_CACHE = nc
    return _NC_CACHE


# ---------------------------------------------------------------- interface
def _install_ntff_hook():
    """The image's antenv lacks axon_hooks; synthesize it so trace=True works."""
    import sys as _sys
    import types

    if "antenv.axon_hooks" in _sys.modules:
        return
    mod = types.ModuleType("antenv.axon_hooks")
    mod._hook = None

    def set_axon_ntff_profile_hook(h):
        mod._hook = h

    def get_axon_ntff_profile_hook():
        return mod._hook

    mod.set_axon_ntff_profile_hook = set_axon_ntff_profile_hook
    mod.get_axon_ntff_profile_hook = get_axon_ntff_profile_hook
    _sys.modules["antenv.axon_hooks"] = mod
    import antenv

    antenv.axon_hooks = mod
    _sys.path.insert(0, "/root/.axon_site")
    from trn_agent_boot.trn_boot import _ntff_profile_via_ctypes

    h = _ntff_profile_via_ctypes("/opt/axon/libaxon_pjrt.so")
    if h is not None:
        set_axon_ntff_profile_hook(h)

    from concourse import bass_utils as _bu

    _bu.upload_artifacts = lambda tmpdir: tmpdir


def _run(inputs_full, trace=False):
    from concourse.bass_utils import run_bass_kernel_spmd

    if trace:
        _install_ntff_hook()

    x = np.asarray(inputs_full, np.float32)
    ident = _host_ident()
    in_maps = [
        {"xs": _host_prep_core(x[i]).reshape(128, C, NFLAT), "ident": ident}
        for i in range(B)
    ]
    nc = _get_nc()
    res = run_bass_kernel_spmd(nc, in_maps, list(range(B)), trace=trace)
    out = np.stack(
        [
            res.results[i]["out"].transpose(0, 2, 3, 1, 4).reshape(C, H, W)
            for i in range(B)
        ],
        axis=0,
    )
    return out, res


def kernel(input):
    out, _ = _run(input, trace=False)
    return out


def kernel_profiled(input):
    return _run(input, trace=True)


# ----------------------------------------------------- local sim validation
def _sim_one(img):
    from concourse.bass_interp import CoreSim

    nc = build_program()
    sim = CoreSim(nc, trace=False)
    sim.tensor("xs")[:] = _host_prep_core(img).reshape(128, C, NFLAT)
    sim.tensor("ident")[:] = _host_ident()
    sim.simulate()
    return np.array(sim.tensor("out")).transpose(0, 2, 3, 1, 4).reshape(C, H, W)


if __name__ == "__main__":
    rng = np.random.default_rng(0)
    img = rng.random((C, H, W), np.float32)
    out = _sim_one(img)
    print("sim out stats", out.min(), out.max(), np.abs(out).mean())


# revision 27
# speedup vs baseline: 1.1393x; 1.1366x over previous
"""Bilateral blur (7x7, sigma_color=0.1, sigma_space=1.5) on 8 trn2 NeuronCores.

Sharding: pure data-parallel, one image (3,512,512) per core.

Layout: each of the 128 partitions owns a 16-row x 128-col region of the
image (p = band*32 + strip). Per channel a partition stores a [28 x 144]
fp16 strip (6-row/6-col halo + alignment junk, owned base at (6,8)), so
every 7x7 tap shift is a pure free-dim offset. A +1-shifted copy (Xo/Wo)
provides 4B-aligned access for odd column shifts (DVE 2x perf mode).

Symmetry: |x(p+t)-x(p)| maps give w_t; the mirrored tap -t reuses the same
map: w_{-t}(p) = w_t(p-t). Only 24 tap-pairs are computed, each on the
union domain owned U owned-t.

Engine split per pair:
  DVE : 3 subtracts, 2 channel-sum adds, 6 weight*pixel mults, acc2 adds
  ACT : 3 abs, Square(sqrt(50)*a), Exp(-g + ln s_t)
  PE  : acc0/acc1 accumulation as identity matmuls into PSUM (f32)
  Pool: wsum accumulation
Center tap is the init (w = s_center exactly: scaled-identity matmul / TS).
Finale: out_c = acc_c * reciprocal(wsum) in fp32, scattered to (3,512,512).
"""

import sys

sys.path.insert(0, "/opt/trn_rl_repo")

import math

import numpy as np

import concourse.bass as bass
import concourse.mybir as mybir
from concourse import tile
from concourse.vector_clock import ScopedClock, VectorClock

# ---------------------------------------------------------------- constants
B, C, H, W = 8, 3, 512, 512
PAD = 3
COLOR_COEF = 50.0  # 0.5 / sigma_color^2

N_BANDS = 4
N_STRIPS = 32
ROWS = 16
COLS = 128
SR = 28  # stored rows: owned rows -6 .. +21
SC = 144  # stored cols: owned cols -8 .. +135
OR0 = 6  # stored row of owned row 0
OC0 = 8  # stored col of owned col 0
NFLAT = SR * SC  # 4032
NOWN = ROWS * COLS  # 2048
MAXW = 2560  # >= (16+3) * 134

F16 = mybir.dt.float16
F32 = mybir.dt.float32

POOL_WSUM = True  # wsum accumulation on the (otherwise idle) Pool engine


def _gaussian1d(ksize, sigma):
    x = np.arange(ksize, dtype=np.float64) - ksize // 2
    g = np.exp(-(x * x) / (2.0 * sigma * sigma))
    return g / g.sum()


_SY = _gaussian1d(7, 1.5)
_S2D = np.outer(_SY, _SY)
S_CENTER = float(_S2D[3, 3])

# 24 representative taps; the mirror -t reuses each w map
PAIRS = [
    (dy, dx)
    for dy in range(0, PAD + 1)
    for dx in range(-PAD, PAD + 1)
    if (dy > 0) or (dy == 0 and dx > 0)
]
assert len(PAIRS) == 24


def _space_w(dy, dx):
    return float(_S2D[dy + 3, dx + 3])


# ------------------------------------------------- tile tail-drain wait fix
# The installed walrus rejects >2 sync-wait commands on one CTRL instruction;
# spread the kernel-tail drain waits across per-proc NOPs (sequential on the
# same engine, so the drain still observes every semaphore target).
def _drain_and_barrier_split(self, tick_clock, wait_clock):
    nc = self.nc
    gc = tick_clock.global_clock
    n = len(gc)
    for proc in range(n):
        if gc[proc] <= 0:
            continue
        vc = VectorClock([gc[p] if p == proc else 0 for p in range(n)])
        nop = nc.sync.nop(nofuse=True, hint=f"tail_wait_p{proc}")
        wait_clock.add_sem_waits(nop.ins, ScopedClock({None: vc}))
    nc.sync.drain()
    nc.all_engine_barrier()
    assert self.sems is not None
    popped = nc._tile_sem_poison_stack.pop()
    assert popped is self._sem_poison
    nc.clear_and_free_semaphores(list(self.sems.allocated().values()))
    nc.all_engine_barrier()


tile.TileContext._drain_and_barrier = _drain_and_barrier_split


def _strip_redundant_self_waits(nc):
    """Remove sem waits that are trivially satisfied by the waiting engine's
    own program order (walrus allows only 1 sync-wait on ACT instructions).

    A wait (S, v) on an instruction of engine E is removable when S is
    incremented exclusively by earlier engine-E instructions whose cumulative
    increment already reaches v by this point in program order."""
    seq = []
    for fn in nc.m.functions:
        for bb in fn.blocks:
            seq.extend(bb.instructions)

    inc_engines = {}
    for ins in seq:
        si = getattr(ins, "sync_info", None)
        if si is None:
            continue
        for u in si.on_update or []:
            inc_engines.setdefault(u.ant_name, set()).add(str(ins.engine))

    cum = {}
    removed = 0
    for ins in seq:
        si = getattr(ins, "sync_info", None)
        if si is None:
            continue
        eng = str(ins.engine)
        ow = si.on_wait or []
        keep = []
        for w in ow:
            engs = inc_engines.get(w.ant_name)
            if (
                engs == {eng}
                and w.wait_mode in ("sem-ge-imm", "sem-ge")
                and cum.get(w.ant_name, 0) >= (w.wait_value or 0)
            ):
                removed += 1
                continue
            keep.append(w)
        if len(keep) != len(ow):
            si.on_wait = keep
        for u in si.on_update or []:
            cum[u.ant_name] = cum.get(u.ant_name, 0) + (u.update_value or 1)
    return removed


def _cap_waits(nc, limit=1):
    """This walrus build encodes at most `limit` sync-waits per instruction.
    Move excess waits backward onto an earlier same-engine instruction j.
    Safe when pos(j) > pos(producer of the waited sem value): the bb order
    is a topological order of the dep DAG, so the producer's transitive
    deps all precede it — no cycle can form; the engine is in-order, so
    waiting at j still gates the original instruction."""
    for fn in nc.m.functions:
        for bb in fn.blocks:
            seq = list(bb.instructions)
            # position where each sem's cumulative inc first reaches a value
            cum = {}
            reach = []  # per position: dict sem -> cum value AFTER this inst
            for ins in seq:
                si = getattr(ins, "sync_info", None)
                if si is not None:
                    for u in si.on_update or []:
                        cum[u.ant_name] = cum.get(u.ant_name, 0) + (
                            u.update_value or 1
                        )
                reach.append(dict(cum))

            def produced_pos(sem, val):
                # first position whose cumulative inc >= val; -1 if already
                # satisfied before this bb
                if val <= 0:
                    return -1
                for p, c in enumerate(reach):
                    if c.get(sem, 0) >= val:
                        return p
                return -1  # producer in an earlier bb

            nwaits = []
            for ins in seq:
                si = getattr(ins, "sync_info", None)
                nwaits.append(len(si.on_wait) if si is not None else 0)

            for i, ins in enumerate(seq):
                si = getattr(ins, "sync_info", None)
                if si is None or nwaits[i] <= limit:
                    continue
                ow = list(si.on_wait)
                # keep the wait with the latest producer (most binding)
                pp = [produced_pos(w.ant_name, w.wait_value or 0) for w in ow]
                order = sorted(range(len(ow)), key=lambda k: pp[k])
                keep_idx = order[-1]
                moved_any = []
                for k in order[:-1]:
                    if len(moved_any) + 1 >= len(ow) - (limit - 1):
                        break  # moved enough
                    w, q = ow[k], pp[k]
                    j = i - 1
                    dst = None
                    while j > q:
                        o = seq[j]
                        osi = getattr(o, "sync_info", None)
                        if (
                            str(o.engine) == str(ins.engine)
                            and osi is not None
                            and nwaits[j] < limit
                        ):
                            dst = j
                            break
                        j -= 1
                    if dst is None:
                        continue
                    osi = seq[dst].sync_info
                    osi.on_wait = list(osi.on_wait) + [w]
                    nwaits[dst] += 1
                    moved_any.append(k)
                if moved_any:
                    si.on_wait = [
                        w for k, w in enumerate(ow) if k not in moved_any
                    ]
                    nwaits[i] = len(si.on_wait)
                if nwaits[i] > limit:
                    raise RuntimeError(
                        f"could not cap waits on {ins.name}: "
                        f"{[(w.ant_name, w.wait_value) for w in si.on_wait]}"
                    )


# ------------------------------------------------------------- host helpers
def _host_prep_core(img):
    """img: (3, 512, 512) f32 -> strips [128, 3, 28, 144] f16."""
    padded = np.pad(img, ((0, 0), (8, 8), (8, 8)), mode="reflect").astype(np.float16)
    out = np.empty((128, C, SR, SC), np.float16)
    for b in range(N_BANDS):
        for s in range(N_STRIPS):
            p = b * N_STRIPS + s
            out[p] = padded[:, 16 * s + 2 : 16 * s + 2 + SR, 128 * b : 128 * b + SC]
    return out


_IDENT = None


def _host_ident():
    global _IDENT
    if _IDENT is None:
        e = np.eye(128, dtype=np.float16)
        _IDENT = np.concatenate([e, -e], 1)
    return _IDENT


def _v3(t, r0, nr, c0, ncols, w):
    ap = t[:]
    nfit = (ap.shape[-1] // w) * w
    if nfit != ap.shape[-1]:
        ap = ap[:, 0:nfit]
    return ap.rearrange("p (r w) -> p r w", w=w)[:, r0 : r0 + nr, c0 : c0 + ncols]


# ------------------------------------------------------------ program build
def build_program():
    nc = bass.Bass()
    xs_in = nc.dram_tensor("xs", [128, C, NFLAT], F16, kind="ExternalInput")
    id_in = nc.dram_tensor("ident", [128, 256], F16, kind="ExternalInput")
    out_d = nc.dram_tensor(
        "out", [C, N_BANDS, N_STRIPS, ROWS, COLS], F32, kind="ExternalOutput"
    )

    alu = mybir.AluOpType
    act = mybir.ActivationFunctionType

    with tile.TileContext(nc) as tc:
        with (
            tc.tile_pool(name="xp", bufs=1) as xp,
            tc.tile_pool(name="wk", bufs=2) as wk,
            tc.tile_pool(name="wp", bufs=3) as wp,
            tc.tile_pool(name="ac", bufs=1) as ac,
            tc.tile_pool(name="ps", bufs=1, space="PSUM") as ps,
        ):
            X = [
                xp.tile([128, NFLAT], F16, tag=f"X{c}", name=f"X{c}") for c in range(C)
            ]
            Xo = [
                xp.tile([128, NFLAT], F16, tag=f"Xo{c}", name=f"Xo{c}")
                for c in range(C)
            ]
            idt = xp.tile([128, 256], F16, tag="idt", name="idt")
            nc.sync.dma_start(idt[:], id_in[:])
            for c in range(C):
                nc.sync.dma_start(X[c][:], xs_in[:, c, :])
            for c in range(C):
                nc.vector.tensor_copy(Xo[c][:, 0 : NFLAT - 1], X[c][:, 1:NFLAT])

            ident = idt[:, 0:128]
            ident_neg = idt[:, 128:256]

            acc2 = ac.tile([128, NOWN], F16, tag="acc2", name="acc2")
            wsum = ac.tile([128, NOWN], F16, tag="wsum", name="wsum")
            pacc = [
                ps.tile([128, NOWN], F32, tag=f"pacc{c}", name=f"pacc{c}")
                for c in range(2)
            ]

            def xsl(c, r0, c0, nr, ncols):
                """strip view at stored rows r0.., cols c0.. (image offsets
                relative to owned origin are handled by callers)."""
                if c0 % 2 == 0:
                    return _v3(X[c], r0, nr, c0, ncols, SC)
                return _v3(Xo[c], r0, nr, c0 - 1, ncols, SC)

            def own(t):  # dense [128,16,128] view of an owned-domain tile
                return _v3(t, 0, ROWS, 0, COLS, COLS)

            # deviation form: n_c = sum_k w_k * (x_c(p+k) - x_c(p)); the
            # center tap contributes nothing, so accumulators start at 0
            # (first matmul start=True; acc2 memset). wsum is split into two
            # independent accumulators (DVE + Pool chains) merged at the end.
            nc.vector.memset(acc2[:], 0.0)
            nc.vector.memset(wsum[:], S_CENTER)
            wsumB = ac.tile([128, NOWN], F16, tag="wsumB", name="wsumB")
            if POOL_WSUM:
                nc.gpsimd.memset(wsumB[:], 0.0)

            # per-pair Exp biases (ln of space weight), via a const fp32 tile
            bias_vals = sorted(
                {round(math.log(_space_w(dy, dx)), 12) for dy, dx in PAIRS}
            )
            bias_idx = {v: i for i, v in enumerate(bias_vals)}
            bias_t = ac.tile([128, len(bias_vals)], F32, tag="bias", name="bias_t")
            for i, v in enumerate(bias_vals):
                nc.vector.memset(bias_t[:, i : i + 1], v)

            def produce_subs(dy, dx):
                """diffs + abs for pair (dy,dx); returns pipeline context."""
                adx = abs(dx)
                qr0, qc0 = -dy, -max(dx, 0)
                qrows, qcols = ROWS + dy, COLS + adx
                wpad = qcols + (qcols & 1)  # even width for 2x alignment

                def wv(t):
                    return _v3(t, 0, qrows, 0, wpad, wpad)

                sr0, sc0 = OR0 + qr0, OC0 + qc0
                ds, abs_ = [], []
                for c in range(C):
                    d = wk.tile(
                        [128, MAXW], F16, tag=f"d{c}", name=f"d{c}", bufs=3
                    )
                    nc.vector.tensor_tensor(
                        wv(d),
                        xsl(c, sr0 + dy, sc0 + dx, qrows, wpad),
                        xsl(c, sr0, sc0, qrows, wpad),
                        alu.subtract,
                    )
                    ab = wk.tile([128, MAXW], F16, tag=f"ab{c}", name=f"ab{c}")
                    nc.scalar.activation(wv(ab), wv(d), act.Abs)
                    ds.append(d)
                    abs_.append(ab)
                return [dy, dx, wpad, ds, abs_, wv, None]

            def produce_rest(ctx):
                """channel-sum + square + exp for a pair."""
                dy, dx, wpad, ds, abs_, wv, _ = ctx
                e = wk.tile([128, MAXW], F16, tag="e", name="e")
                nc.vector.tensor_tensor(wv(e), wv(abs_[0]), wv(abs_[1]), alu.add)
                nc.vector.tensor_tensor(wv(e), wv(e), wv(abs_[2]), alu.add)
                nc.scalar.activation(
                    wv(e), wv(e), act.Square, scale=math.sqrt(COLOR_COEF)
                )
                wt = wp.tile([128, MAXW], F16, tag="wt", name="wt")
                bi = bias_idx[round(math.log(_space_w(dy, dx)), 12)]
                nc.scalar.activation(
                    wv(wt), wv(e), act.Exp, bias=bias_t[:, bi : bi + 1], scale=-1.0
                )
                ctx[6] = wt

            def consume_mults(ctx, first, last):
                dy, dx, wpad, ds, abs_, wv, wt = ctx
                cA, cB = max(dx, 0), max(-dx, 0)
                # m_c = w_t * d_c in-place on d_c; read at (dy,cA) for +t and
                # negated at (0,cB) for -t
                for c in range(C):
                    nc.vector.tensor_tensor(wv(ds[c]), wv(ds[c]), wv(wt), alu.mult)

                def mview(c, r0, c0, k):
                    return _v3(ds[c], r0 + 4 * k, 4, c0, COLS, wpad)

                for c in range(2):
                    for k in range(4):
                        sl = slice(512 * k, 512 * (k + 1))
                        nc.tensor.matmul(
                            pacc[c][:, sl], ident, mview(c, dy, cA, k),
                            start=first, stop=False, skip_group_check=True,
                        )
                        nc.tensor.matmul(
                            pacc[c][:, sl], ident_neg, mview(c, 0, cB, k),
                            start=False, stop=last, skip_group_check=True,
                        )

            def consume_rest(ctx):
                dy, dx, wpad, ds, abs_, wv, wt = ctx
                cA, cB = max(dx, 0), max(-dx, 0)
                m2p = _v3(ds[2], dy, ROWS, cA, COLS, wpad)
                m2m = _v3(ds[2], 0, ROWS, cB, COLS, wpad)
                nc.vector.tensor_tensor(own(acc2), own(acc2), m2p, alu.add)
                nc.vector.tensor_tensor(own(acc2), own(acc2), m2m, alu.subtract)

                wA = _v3(wt, dy, ROWS, cA, COLS, wpad)
                wB = _v3(wt, 0, ROWS, cB, COLS, wpad)
                nc.vector.tensor_tensor(own(wsum), own(wsum), wA, alu.add)
                if POOL_WSUM:
                    nc.gpsimd.tensor_tensor(own(wsumB), own(wsumB), wB, alu.add)
                else:
                    nc.vector.tensor_tensor(own(wsum), own(wsum), wB, alu.add)

            # software pipeline: produce pair i+1 fully, then consume pair i
            prev = None
            for pi, (dy, dx) in enumerate(PAIRS):
                cur = produce_subs(dy, dx)
                produce_rest(cur)
                if prev is not None:
                    consume_mults(prev, first=(pi == 1), last=False)
                    consume_rest(prev)
                prev = cur
            consume_mults(prev, first=False, last=True)
            consume_rest(prev)

            # ---- finale: out_c = x_c + n_c * (1 / wsum) ----
            if POOL_WSUM:
                nc.vector.tensor_tensor(wsum[:], wsum[:], wsumB[:], alu.add)
            rec = ac.tile([128, NOWN], F32, tag="rec", name="rec")
            nc.vector.reciprocal(rec[:], wsum[:])
            outs = []
            for c in range(C):
                o = ac.tile([128, NOWN], F32, tag="o", name=f"oo{c}", bufs=3)
                src = pacc[c][:] if c < 2 else acc2[:]
                nc.vector.tensor_tensor(o[:], src, rec[:], alu.mult)
                nc.vector.tensor_tensor(
                    own(o), own(o), xsl(c, OR0, OC0, ROWS, COLS), alu.add
                )
                outs.append(o)
            for c in range(C):
                nc.sync.dma_start(
                    out_d[c].rearrange("b s r w -> (b s) (r w)"), outs[c][:]
                )
    return nc


_NC_CACHE = None


def _get_nc():
    global _NC_CACHE
    if _NC_CACHE is None:
        nc = build_program()
        # walrus in this image encodes at most 1 sync-wait per instruction;
        # rely on same-engine program order instead (safe on in-order engines)
        _strip_redundant_self_waits(nc)
        _cap_waits(nc, limit=1)
        _NC_CACHE = nc
    return _NC_CACHE


# ---------------------------------------------------------------- interface
def _install_ntff_hook():
    """The image's antenv lacks axon_hooks; synthesize it so trace=True works."""
    import sys as _sys
    import types

    if "antenv.axon_hooks" in _sys.modules:
        return
    mod = types.ModuleType("antenv.axon_hooks")
    mod._hook = None

    def set_axon_ntff_profile_hook(h):
        mod._hook = h

    def get_axon_ntff_profile_hook():
        return mod._hook

    mod.set_axon_ntff_profile_hook = set_axon_ntff_profile_hook
    mod.get_axon_ntff_profile_hook = get_axon_ntff_profile_hook
    _sys.modules["antenv.axon_hooks"] = mod
    import antenv

    antenv.axon_hooks = mod
    _sys.path.insert(0, "/root/.axon_site")
    from trn_agent_boot.trn_boot import _ntff_profile_via_ctypes

    h = _ntff_profile_via_ctypes("/opt/axon/libaxon_pjrt.so")
    if h is not None:
        set_axon_ntff_profile_hook(h)

    from concourse import bass_utils as _bu

    _bu.upload_artifacts = lambda tmpdir: tmpdir


def _run(inputs_full, trace=False):
    from concourse.bass_utils import run_bass_kernel_spmd

    if trace:
        _install_ntff_hook()

    x = np.asarray(inputs_full, np.float32)
    ident = _host_ident()
    in_maps = [
        {"xs": _host_prep_core(x[i]).reshape(128, C, NFLAT), "ident": ident}
        for i in range(B)
    ]
    nc = _get_nc()
    res = run_bass_kernel_spmd(nc, in_maps, list(range(B)), trace=trace)
    out = np.stack(
        [
            res.results[i]["out"].transpose(0, 2, 3, 1, 4).reshape(C, H, W)
            for i in range(B)
        ],
        axis=0,
    )
    return out, res


def kernel(input):
    out, _ = _run(input, trace=False)
    return out


def kernel_profiled(input):
    return _run(input, trace=True)


# ----------------------------------------------------- local sim validation
def _sim_one(img):
    from concourse.bass_interp import CoreSim

    nc = build_program()
    sim = CoreSim(nc, trace=False)
    sim.tensor("xs")[:] = _host_prep_core(img).reshape(128, C, NFLAT)
    sim.tensor("ident")[:] = _host_ident()
    sim.simulate()
    return np.array(sim.tensor("out")).transpose(0, 2, 3, 1, 4).reshape(C, H, W)


if __name__ == "__main__":
    rng = np.random.default_rng(0)
    img = rng.random((C, H, W), np.float32)
    out = _sim_one(img)
    print("sim out stats", out.min(), out.max(), np.abs(out).mean())


# revision 28
# speedup vs baseline: 1.3120x; 1.1516x over previous
"""Bilateral blur (7x7, sigma_color=0.1, sigma_space=1.5) on 8 trn2 NeuronCores.

Sharding: pure data-parallel, one image (3,512,512) per core.

Layout: each of the 128 partitions owns a 16-row x 128-col region of the
image (p = band*32 + strip). Per channel a partition stores a [28 x 144]
fp16 strip (6-row/6-col halo + alignment junk, owned base at (6,8)), so
every 7x7 tap shift is a pure free-dim offset. A +1-shifted copy (Xo/Wo)
provides 4B-aligned access for odd column shifts (DVE 2x perf mode).

Symmetry: |x(p+t)-x(p)| maps give w_t; the mirrored tap -t reuses the same
map: w_{-t}(p) = w_t(p-t). Only 24 tap-pairs are computed, each on the
union domain owned U owned-t.

Engine split per pair:
  DVE : 3 subtracts, 2 channel-sum adds, 6 weight*pixel mults, acc2 adds
  ACT : 3 abs, Square(sqrt(50)*a), Exp(-g + ln s_t)
  PE  : acc0/acc1 accumulation as identity matmuls into PSUM (f32)
  Pool: wsum accumulation
Center tap is the init (w = s_center exactly: scaled-identity matmul / TS).
Finale: out_c = acc_c * reciprocal(wsum) in fp32, scattered to (3,512,512).
"""

import sys

sys.path.insert(0, "/opt/trn_rl_repo")

import math

import numpy as np

import concourse.bass as bass
import concourse.mybir as mybir
from concourse import tile
from concourse.vector_clock import ScopedClock, VectorClock

# ---------------------------------------------------------------- constants
B, C, H, W = 8, 3, 512, 512
PAD = 3
COLOR_COEF = 50.0  # 0.5 / sigma_color^2

N_BANDS = 4
N_STRIPS = 32
ROWS = 16
COLS = 128
SR = 28  # stored rows: owned rows -6 .. +21
SC = 144  # stored cols: owned cols -8 .. +135
OR0 = 6  # stored row of owned row 0
OC0 = 8  # stored col of owned col 0
NFLAT = SR * SC  # 4032
NOWN = ROWS * COLS  # 2048
MAXW = 2560  # >= (16+3) * 134

F16 = mybir.dt.float16
F32 = mybir.dt.float32

POOL_WSUM = False  # Pool TT blocks the shared SBUF port and stalls DVE


def _gaussian1d(ksize, sigma):
    x = np.arange(ksize, dtype=np.float64) - ksize // 2
    g = np.exp(-(x * x) / (2.0 * sigma * sigma))
    return g / g.sum()


_SY = _gaussian1d(7, 1.5)
_S2D = np.outer(_SY, _SY)
S_CENTER = float(_S2D[3, 3])

# 24 representative taps; the mirror -t reuses each w map
PAIRS = [
    (dy, dx)
    for dy in range(0, PAD + 1)
    for dx in range(-PAD, PAD + 1)
    if (dy > 0) or (dy == 0 and dx > 0)
]
assert len(PAIRS) == 24


def _space_w(dy, dx):
    return float(_S2D[dy + 3, dx + 3])


# ------------------------------------------------- tile tail-drain wait fix
# The installed walrus rejects >2 sync-wait commands on one CTRL instruction;
# spread the kernel-tail drain waits across per-proc NOPs (sequential on the
# same engine, so the drain still observes every semaphore target).
def _drain_and_barrier_split(self, tick_clock, wait_clock):
    nc = self.nc
    gc = tick_clock.global_clock
    n = len(gc)
    for proc in range(n):
        if gc[proc] <= 0:
            continue
        vc = VectorClock([gc[p] if p == proc else 0 for p in range(n)])
        nop = nc.sync.nop(nofuse=True, hint=f"tail_wait_p{proc}")
        wait_clock.add_sem_waits(nop.ins, ScopedClock({None: vc}))
    nc.sync.drain()
    nc.all_engine_barrier()
    assert self.sems is not None
    popped = nc._tile_sem_poison_stack.pop()
    assert popped is self._sem_poison
    nc.clear_and_free_semaphores(list(self.sems.allocated().values()))
    nc.all_engine_barrier()


tile.TileContext._drain_and_barrier = _drain_and_barrier_split


def _strip_redundant_self_waits(nc):
    """Remove sem waits that are trivially satisfied by the waiting engine's
    own program order (walrus allows only 1 sync-wait on ACT instructions).

    A wait (S, v) on an instruction of engine E is removable when S is
    incremented exclusively by earlier engine-E instructions whose cumulative
    increment already reaches v by this point in program order."""
    seq = []
    for fn in nc.m.functions:
        for bb in fn.blocks:
            seq.extend(bb.instructions)

    inc_engines = {}
    for ins in seq:
        si = getattr(ins, "sync_info", None)
        if si is None:
            continue
        for u in si.on_update or []:
            inc_engines.setdefault(u.ant_name, set()).add(str(ins.engine))

    cum = {}
    removed = 0
    for ins in seq:
        si = getattr(ins, "sync_info", None)
        if si is None:
            continue
        eng = str(ins.engine)
        ow = si.on_wait or []
        keep = []
        for w in ow:
            engs = inc_engines.get(w.ant_name)
            if (
                engs == {eng}
                and w.wait_mode in ("sem-ge-imm", "sem-ge")
                and cum.get(w.ant_name, 0) >= (w.wait_value or 0)
            ):
                removed += 1
                continue
            keep.append(w)
        if len(keep) != len(ow):
            si.on_wait = keep
        for u in si.on_update or []:
            cum[u.ant_name] = cum.get(u.ant_name, 0) + (u.update_value or 1)
    return removed


def _cap_waits(nc, limit=1):
    """This walrus build encodes at most `limit` sync-waits per instruction.
    Move excess waits backward onto an earlier same-engine instruction j.
    Safe when pos(j) > pos(producer of the waited sem value): the bb order
    is a topological order of the dep DAG, so the producer's transitive
    deps all precede it — no cycle can form; the engine is in-order, so
    waiting at j still gates the original instruction."""
    for fn in nc.m.functions:
        for bb in fn.blocks:
            seq = list(bb.instructions)
            # position where each sem's cumulative inc first reaches a value
            cum = {}
            reach = []  # per position: dict sem -> cum value AFTER this inst
            for ins in seq:
                si = getattr(ins, "sync_info", None)
                if si is not None:
                    for u in si.on_update or []:
                        cum[u.ant_name] = cum.get(u.ant_name, 0) + (
                            u.update_value or 1
                        )
                reach.append(dict(cum))

            def produced_pos(sem, val):
                # first position whose cumulative inc >= val; -1 if already
                # satisfied before this bb
                if val <= 0:
                    return -1
                for p, c in enumerate(reach):
                    if c.get(sem, 0) >= val:
                        return p
                return -1  # producer in an earlier bb

            nwaits = []
            for ins in seq:
                si = getattr(ins, "sync_info", None)
                nwaits.append(len(si.on_wait) if si is not None else 0)

            for i, ins in enumerate(seq):
                si = getattr(ins, "sync_info", None)
                if si is None or nwaits[i] <= limit:
                    continue
                ow = list(si.on_wait)
                # keep the wait with the latest producer (most binding)
                pp = [produced_pos(w.ant_name, w.wait_value or 0) for w in ow]
                order = sorted(range(len(ow)), key=lambda k: pp[k])
                keep_idx = order[-1]
                moved_any = []
                for k in order[:-1]:
                    if len(moved_any) + 1 >= len(ow) - (limit - 1):
                        break  # moved enough
                    w, q = ow[k], pp[k]
                    j = i - 1
                    dst = None
                    while j > q:
                        o = seq[j]
                        osi = getattr(o, "sync_info", None)
                        if (
                            str(o.engine) == str(ins.engine)
                            and osi is not None
                            and nwaits[j] < limit
                        ):
                            dst = j
                            break
                        j -= 1
                    if dst is None:
                        continue
                    osi = seq[dst].sync_info
                    osi.on_wait = list(osi.on_wait) + [w]
                    nwaits[dst] += 1
                    moved_any.append(k)
                if moved_any:
                    si.on_wait = [
                        w for k, w in enumerate(ow) if k not in moved_any
                    ]
                    nwaits[i] = len(si.on_wait)
                if nwaits[i] > limit:
                    raise RuntimeError(
                        f"could not cap waits on {ins.name}: "
                        f"{[(w.ant_name, w.wait_value) for w in si.on_wait]}"
                    )


# ------------------------------------------------------------- host helpers
def _host_prep_core(img):
    """img: (3, 512, 512) f32 -> strips [128, 3, 28, 144] f16."""
    padded = np.pad(img, ((0, 0), (8, 8), (8, 8)), mode="reflect").astype(np.float16)
    out = np.empty((128, C, SR, SC), np.float16)
    for b in range(N_BANDS):
        for s in range(N_STRIPS):
            p = b * N_STRIPS + s
            out[p] = padded[:, 16 * s + 2 : 16 * s + 2 + SR, 128 * b : 128 * b + SC]
    return out


_IDENT = None


def _host_ident():
    global _IDENT
    if _IDENT is None:
        e = np.eye(128, dtype=np.float16)
        _IDENT = np.concatenate([e, -e], 1)
    return _IDENT


def _v3(t, r0, nr, c0, ncols, w):
    ap = t[:]
    nfit = (ap.shape[-1] // w) * w
    if nfit != ap.shape[-1]:
        ap = ap[:, 0:nfit]
    return ap.rearrange("p (r w) -> p r w", w=w)[:, r0 : r0 + nr, c0 : c0 + ncols]


# ------------------------------------------------------------ program build
def build_program():
    nc = bass.Bass()
    xs_in = nc.dram_tensor("xs", [128, C, NFLAT], F16, kind="ExternalInput")
    id_in = nc.dram_tensor("ident", [128, 256], F16, kind="ExternalInput")
    out_d = nc.dram_tensor(
        "out", [C, N_BANDS, N_STRIPS, ROWS, COLS], F32, kind="ExternalOutput"
    )

    alu = mybir.AluOpType
    act = mybir.ActivationFunctionType

    with tile.TileContext(nc) as tc:
        with (
            tc.tile_pool(name="xp", bufs=1) as xp,
            tc.tile_pool(name="wk", bufs=2) as wk,
            tc.tile_pool(name="wp", bufs=3) as wp,
            tc.tile_pool(name="ac", bufs=1) as ac,
            tc.tile_pool(name="ps", bufs=1, space="PSUM") as ps,
        ):
            X = [
                xp.tile([128, NFLAT], F16, tag=f"X{c}", name=f"X{c}") for c in range(C)
            ]
            Xo = [
                xp.tile([128, NFLAT], F16, tag=f"Xo{c}", name=f"Xo{c}")
                for c in range(C)
            ]
            idt = xp.tile([128, 256], F16, tag="idt", name="idt")
            nc.sync.dma_start(idt[:], id_in[:])
            for c in range(C):
                nc.sync.dma_start(X[c][:], xs_in[:, c, :])
            for c in range(C):
                nc.vector.tensor_copy(Xo[c][:, 0 : NFLAT - 1], X[c][:, 1:NFLAT])

            ident = idt[:, 0:128]
            ident_neg = idt[:, 128:256]

            acc2 = ac.tile([128, NOWN], F16, tag="acc2", name="acc2")
            wsum = ac.tile([128, NOWN], F16, tag="wsum", name="wsum")
            pacc = [
                ps.tile([128, NOWN], F32, tag=f"pacc{c}", name=f"pacc{c}")
                for c in range(2)
            ]

            def xsl(c, r0, c0, nr, ncols):
                """strip view at stored rows r0.., cols c0.. (image offsets
                relative to owned origin are handled by callers)."""
                if c0 % 2 == 0:
                    return _v3(X[c], r0, nr, c0, ncols, SC)
                return _v3(Xo[c], r0, nr, c0 - 1, ncols, SC)

            def own(t):  # dense [128,16,128] view of an owned-domain tile
                return _v3(t, 0, ROWS, 0, COLS, COLS)

            # deviation form: n_c = sum_k w_k * (x_c(p+k) - x_c(p)); the
            # center tap contributes nothing, so accumulators start at 0
            # (first matmul start=True; acc2 memset). wsum is split into two
            # independent accumulators (DVE + Pool chains) merged at the end.
            nc.vector.memset(acc2[:], 0.0)
            nc.vector.memset(wsum[:], S_CENTER)
            wsumB = ac.tile([128, NOWN], F16, tag="wsumB", name="wsumB")
            if POOL_WSUM:
                nc.gpsimd.memset(wsumB[:], 0.0)

            # per-pair Exp biases (ln of space weight), via a const fp32 tile
            bias_vals = sorted(
                {round(math.log(_space_w(dy, dx)), 12) for dy, dx in PAIRS}
            )
            bias_idx = {v: i for i, v in enumerate(bias_vals)}
            bias_t = ac.tile([128, len(bias_vals)], F32, tag="bias", name="bias_t")
            for i, v in enumerate(bias_vals):
                nc.vector.memset(bias_t[:, i : i + 1], v)

            def produce_subs(dy, dx):
                """diffs + abs for pair (dy,dx); returns pipeline context."""
                adx = abs(dx)
                qr0, qc0 = -dy, -max(dx, 0)
                qrows, qcols = ROWS + dy, COLS + adx
                wpad = qcols + (qcols & 1)  # even width for 2x alignment

                def wv(t):
                    return _v3(t, 0, qrows, 0, wpad, wpad)

                sr0, sc0 = OR0 + qr0, OC0 + qc0
                ds, abs_ = [], []
                for c in range(C):
                    d = wk.tile(
                        [128, MAXW], F16, tag=f"d{c}", name=f"d{c}", bufs=3
                    )
                    nc.vector.tensor_tensor(
                        wv(d),
                        xsl(c, sr0 + dy, sc0 + dx, qrows, wpad),
                        xsl(c, sr0, sc0, qrows, wpad),
                        alu.subtract,
                    )
                    ab = wk.tile([128, MAXW], F16, tag=f"ab{c}", name=f"ab{c}")
                    nc.scalar.activation(wv(ab), wv(d), act.Abs)
                    ds.append(d)
                    abs_.append(ab)
                return [dy, dx, wpad, ds, abs_, wv, None]

            def produce_rest(ctx):
                """channel-sum + square + exp for a pair."""
                dy, dx, wpad, ds, abs_, wv, _ = ctx
                e = wk.tile([128, MAXW], F16, tag="e", name="e")
                nc.vector.tensor_tensor(wv(e), wv(abs_[0]), wv(abs_[1]), alu.add)
                nc.vector.tensor_tensor(wv(e), wv(e), wv(abs_[2]), alu.add)
                nc.scalar.activation(
                    wv(e), wv(e), act.Square, scale=math.sqrt(COLOR_COEF)
                )
                wt = wp.tile([128, MAXW], F16, tag="wt", name="wt")
                bi = bias_idx[round(math.log(_space_w(dy, dx)), 12)]
                nc.scalar.activation(
                    wv(wt), wv(e), act.Exp, bias=bias_t[:, bi : bi + 1], scale=-1.0
                )
                ctx[6] = wt

            def consume_mults(ctx, first, last):
                dy, dx, wpad, ds, abs_, wv, wt = ctx
                cA, cB = max(dx, 0), max(-dx, 0)
                # m_c = w_t * d_c in-place on d_c; read at (dy,cA) for +t and
                # negated at (0,cB) for -t
                for c in range(C):
                    nc.vector.tensor_tensor(wv(ds[c]), wv(ds[c]), wv(wt), alu.mult)

                def mview(c, r0, c0, k):
                    return _v3(ds[c], r0 + 4 * k, 4, c0, COLS, wpad)

                for c in range(2):
                    for k in range(4):
                        sl = slice(512 * k, 512 * (k + 1))
                        nc.tensor.matmul(
                            pacc[c][:, sl], ident, mview(c, dy, cA, k),
                            start=first, stop=False, skip_group_check=True,
                        )
                        nc.tensor.matmul(
                            pacc[c][:, sl], ident_neg, mview(c, 0, cB, k),
                            start=False, stop=last, skip_group_check=True,
                        )

            def consume_rest(ctx):
                dy, dx, wpad, ds, abs_, wv, wt = ctx
                cA, cB = max(dx, 0), max(-dx, 0)
                m2p = _v3(ds[2], dy, ROWS, cA, COLS, wpad)
                m2m = _v3(ds[2], 0, ROWS, cB, COLS, wpad)
                nc.vector.tensor_tensor(own(acc2), own(acc2), m2p, alu.add)
                nc.vector.tensor_tensor(own(acc2), own(acc2), m2m, alu.subtract)

                wA = _v3(wt, dy, ROWS, cA, COLS, wpad)
                wB = _v3(wt, 0, ROWS, cB, COLS, wpad)
                nc.vector.tensor_tensor(own(wsum), own(wsum), wA, alu.add)
                if POOL_WSUM:
                    nc.gpsimd.tensor_tensor(own(wsumB), own(wsumB), wB, alu.add)
                else:
                    nc.vector.tensor_tensor(own(wsum), own(wsum), wB, alu.add)

            # software pipeline: produce pair i+1 fully, then consume pair i
            prev = None
            for pi, (dy, dx) in enumerate(PAIRS):
                cur = produce_subs(dy, dx)
                produce_rest(cur)
                if prev is not None:
                    consume_mults(prev, first=(pi == 1), last=False)
                    consume_rest(prev)
                prev = cur
            consume_mults(prev, first=False, last=True)
            consume_rest(prev)

            # ---- finale: out_c = x_c + n_c * (1 / wsum) ----
            if POOL_WSUM:
                nc.vector.tensor_tensor(wsum[:], wsum[:], wsumB[:], alu.add)
            rec = ac.tile([128, NOWN], F32, tag="rec", name="rec")
            nc.vector.reciprocal(rec[:], wsum[:])
            outs = []
            for c in range(C):
                o = ac.tile([128, NOWN], F32, tag="o", name=f"oo{c}", bufs=3)
                src = pacc[c][:] if c < 2 else acc2[:]
                nc.vector.tensor_tensor(o[:], src, rec[:], alu.mult)
                nc.vector.tensor_tensor(
                    own(o), own(o), xsl(c, OR0, OC0, ROWS, COLS), alu.add
                )
                outs.append(o)
            for c in range(C):
                nc.sync.dma_start(
                    out_d[c].rearrange("b s r w -> (b s) (r w)"), outs[c][:]
                )
    return nc


_NC_CACHE = None


def _get_nc():
    global _NC_CACHE
    if _NC_CACHE is None:
        nc = build_program()
        # walrus in this image encodes at most 1 sync-wait per instruction;
        # rely on same-engine program order instead (safe on in-order engines)
        _strip_redundant_self_waits(nc)
        _cap_waits(nc, limit=1)
        _NC_CACHE = nc
    return _NC_CACHE


# ---------------------------------------------------------------- interface
def _install_ntff_hook():
    """The image's antenv lacks axon_hooks; synthesize it so trace=True works."""
    import sys as _sys
    import types

    if "antenv.axon_hooks" in _sys.modules:
        return
    mod = types.ModuleType("antenv.axon_hooks")
    mod._hook = None

    def set_axon_ntff_profile_hook(h):
        mod._hook = h

    def get_axon_ntff_profile_hook():
        return mod._hook

    mod.set_axon_ntff_profile_hook = set_axon_ntff_profile_hook
    mod.get_axon_ntff_profile_hook = get_axon_ntff_profile_hook
    _sys.modules["antenv.axon_hooks"] = mod
    import antenv

    antenv.axon_hooks = mod
    _sys.path.insert(0, "/root/.axon_site")
    from trn_agent_boot.trn_boot import _ntff_profile_via_ctypes

    h = _ntff_profile_via_ctypes("/opt/axon/libaxon_pjrt.so")
    if h is not None:
        set_axon_ntff_profile_hook(h)

    from concourse import bass_utils as _bu

    _bu.upload_artifacts = lambda tmpdir: tmpdir


def _run(inputs_full, trace=False):
    from concourse.bass_utils import run_bass_kernel_spmd

    if trace:
        _install_ntff_hook()

    x = np.asarray(inputs_full, np.float32)
    ident = _host_ident()
    in_maps = [
        {"xs": _host_prep_core(x[i]).reshape(128, C, NFLAT), "ident": ident}
        for i in range(B)
    ]
    nc = _get_nc()
    res = run_bass_kernel_spmd(nc, in_maps, list(range(B)), trace=trace)
    out = np.stack(
        [
            res.results[i]["out"].transpose(0, 2, 3, 1, 4).reshape(C, H, W)
            for i in range(B)
        ],
        axis=0,
    )
    return out, res


def kernel(input):
    out, _ = _run(input, trace=False)
    return out


def kernel_profiled(input):
    return _run(input, trace=True)


# ----------------------------------------------------- local sim validation
def _sim_one(img):
    from concourse.bass_interp import CoreSim

    nc = build_program()
    sim = CoreSim(nc, trace=False)
    sim.tensor("xs")[:] = _host_prep_core(img).reshape(128, C, NFLAT)
    sim.tensor("ident")[:] = _host_ident()
    sim.simulate()
    return np.array(sim.tensor("out")).transpose(0, 2, 3, 1, 4).reshape(C, H, W)


if __name__ == "__main__":
    rng = np.random.default_rng(0)
    img = rng.random((C, H, W), np.float32)
    out = _sim_one(img)
    print("sim out stats", out.min(), out.max(), np.abs(out).mean())
